# revision 1
# baseline (speedup 1.0000x reference)
"""Trainium2 Bass kernel for nn_DAE_44779329028610 (embedding autoencoder).

  y = sigmoid(sigmoid(x @ w + b) @ w.T)
  x [4096, 81616] f32, w [81616, 32] f32, b [32] f32 -> y [4096, 81616] f32

Strategy: data-parallel shard of the batch dim across 8 NeuronCores
(512 rows/core); w and b replicated. Per core, a two-pass Tile kernel:

Pass 1 (encoder): stream x in [128, 2048]-tiles (one 4 MiB DMA covers all
four batch tiles of a vocab super-chunk); PE-transpose each 128x128 block
(f32 transpose-mode matmuls, identity stationary) into PSUM; evict
PSUM->SBUF with an f32->f32r rounding copy (walrus requires fp32r matmul
operands to come from an explicit rounding producer); accumulate
hT[32, 512] in one PSUM bank over all 638 vocab chunks with the w-chunk
[128v, 32] (also rounded to f32r) as the stationary. fp32r matmuls run at
1 cycle/row for N>=256 vs 4 for plain f32, at ~1e-4 relative precision.
Each w-chunk is additionally PE-transposed, staged in SBUF, and DMA'd
into a persistent wT store laid out in 4 partition groups (vocab quarter
g lives on partitions 32g..32g+32) for pass 2.

Pass 2 (decoder): hT (sigmoid+bias applied on ACT while reading PSUM,
rounded to f32r, replicated to all 4 partition groups by SBUF->SBUF DMA)
is the stationary [32, 128]; wT chunks [32, 512] are the moving operand;
K=32 matmuls run in PE row group g (tile_position=(32g, 0)); ACT applies
sigmoid PSUM->SBUF; y leaves in [128, 4096]-tiles (2 MiB DMAs).

The workload is HBM-bound (read 1.34 GB of x + write 1.34 GB of y); the
kernel keeps every compute engine well under the per-core DMA time
(~0.5 ms in + ~0.5 ms out at ~360 GB/s), reads w only once, and never
spills intermediates to DRAM.
"""

import sys

if "/opt/trn_rl_repo" not in sys.path:
    sys.path.insert(0, "/opt/trn_rl_repo")

from contextlib import ExitStack

import numpy as np

from concourse import bacc, masks, mybir, tile
from concourse.bass_utils import run_bass_kernel_spmd

# The neuronx_cc hook recompiles the NEFF from scratch in every process
# (~5 min of walrus for this kernel). Cache the compiled NEFF on disk,
# keyed by the BIR hash, so repeat runs are instant.
import hashlib
import os
import shutil

import concourse.bass2jax as _bass2jax

_NEFF_CACHE_DIR = "/tmp/bass_neff_cache"
_orig_compile_bir_kernel = _bass2jax.compile_bir_kernel


def _cached_compile_bir_kernel(bir_json, tmpdir, neff_name="file.neff"):
    os.makedirs(_NEFF_CACHE_DIR, exist_ok=True)
    key = hashlib.sha256(bir_json).hexdigest()[:32]
    cpath = os.path.join(_NEFF_CACHE_DIR, f"{key}.neff")
    out = os.path.join(tmpdir, neff_name)
    if os.path.exists(cpath):
        shutil.copyfile(cpath, out)
        return out
    out = _orig_compile_bir_kernel(bir_json, tmpdir, neff_name)
    try:
        shutil.copyfile(out, cpath)
    except OSError:
        pass
    return out


_bass2jax.compile_bir_kernel = _cached_compile_bir_kernel

F32 = mybir.dt.float32
F32R = mybir.dt.float32r

B_FULL = 4096
V = 81616
D = 32
N_CORES = 8
B_CORE = B_FULL // N_CORES


def _ceil_div(a, b):
    return -(-a // b)


def build_dae(B_core, V, S=2048, y_tile_chunks=8, x_bufs=2, xt_bufs=3, y_bufs=4, repeat=1):
    """Build + compile the per-core Bass program. S = vocab super-chunk."""
    assert B_core % 128 == 0
    nbt = B_core // 128  # batch tiles per core
    NB = nbt * 128
    assert NB <= 512
    assert S % 128 == 0

    # vocab quarters (wT partition groups), multiples of 512
    qb = max(512, _ceil_div(_ceil_div(V, 4), 512) * 512)
    quarters = []  # (start, size)
    for g in range(4):
        s = min(g * qb, V)
        e = min((g + 1) * qb, V)
        quarters.append((s, e - s))

    nc = bacc.Bacc("TRN2", target_bir_lowering=False, debug=False)

    x_d = nc.dram_tensor("x", [B_core, V], F32, kind="ExternalInput")
    w_d = nc.dram_tensor("w", [V, D], F32, kind="ExternalInput")
    b_d = nc.dram_tensor("b", [D], F32, kind="ExternalInput")
    y_d = nc.dram_tensor("y", [B_core, V], F32, kind="ExternalOutput")

    n_chunks_total = _ceil_div(V, 128)  # encoder 128-chunks

    with tile.TileContext(nc) as tc, ExitStack() as ctx:
        const_pool = ctx.enter_context(tc.tile_pool(name="const", bufs=1))
        ident = const_pool.tile([128, 128], F32)
        masks.make_identity(nc, ident[:])
        b_sb = const_pool.tile([D, 1], F32)
        nc.sync.dma_start(b_sb[:, 0:1], b_d[:].unsqueeze(1))

        # persistent stores: wT quarters + replicated hT, both f32r
        wt_pool = ctx.enter_context(tc.tile_pool(name="wt", bufs=1))
        wT = wt_pool.tile([128, qb], F32R)
        hT_rep = wt_pool.tile([128, NB], F32R)

        def _passes():
            # ---------------- pass 1: encoder ----------------
            with ExitStack() as p1:
                xpool = p1.enter_context(tc.tile_pool(name="x", bufs=x_bufs))
                wpool = p1.enter_context(tc.tile_pool(name="w", bufs=2))
                xtpool = p1.enter_context(tc.tile_pool(name="xt", bufs=xt_bufs))
                ps_x = p1.enter_context(tc.tile_pool(name="psx", bufs=3, space="PSUM"))
                ps_w = p1.enter_context(tc.tile_pool(name="psw", bufs=2, space="PSUM"))
                ps_h = p1.enter_context(tc.tile_pool(name="psh", bufs=1, space="PSUM"))

                hT_ps = ps_h.tile([D, NB], F32)

                chunk_i = 0  # global 128-chunk index
                for v0 in range(0, V, S):
                    sl = min(S, V - v0)  # super-chunk len
                    n_sub = _ceil_div(sl, 128)
                    # x: one DMA for all batch tiles of this super-chunk
                    x_t = xpool.tile([128, nbt, sl], F32)
                    nc.sync.dma_start(
                        x_t[:], x_d[:, v0 : v0 + sl].rearrange("(t p) v -> p t v", p=128)
                    )
                    # w rows v0:v0+sl scattered as [128, n_sub, D]
                    w_t = wpool.tile([128, n_sub, D], F32)
                    nfull = sl // 128
                    rem = sl - nfull * 128
                    if nfull:
                        nc.sync.dma_start(
                            w_t[:, 0:nfull, :],
                            w_d[v0 : v0 + nfull * 128, :].rearrange(
                                "(c p) d -> p c d", p=128
                            ),
                        )
                    if rem:
                        nc.sync.dma_start(
                            w_t[0:rem, nfull, :],
                            w_d[v0 + nfull * 128 : v0 + sl, :],
                        )
                    # rounded copy of w for f32r matmuls (written regions only)
                    w_r = wpool.tile([128, n_sub, D], F32R, tag="w_r")
                    if nfull:
                        nc.scalar.copy(w_r[:, 0:nfull, :], w_t[:, 0:nfull, :])
                    if rem:
                        nc.scalar.copy(w_r[0:rem, nfull, :], w_t[0:rem, nfull, :])
                    # staging tile for this super-chunk's slice of wT (f32r,
                    # partitions 0..32; DMA'd to the right partition group below)
                    wt_stage = wpool.tile([D, S], F32R, tag="wt_stage")
                    for c in range(n_sub):
                        vlen = min(128, sl - c * 128)
                        # transpose nbt x-blocks into one psum tile (plain f32)
                        xT_ps = ps_x.tile([128, NB], F32)
                        for t in range(nbt):
                            nc.tensor.matmul(
                                xT_ps[0:vlen, t * 128 : (t + 1) * 128],
                                x_t[:, t, c * 128 : c * 128 + vlen],
                                ident[:, 0:128],
                                is_transpose=True,
                            )
                        # evict + round to f32r
                        xT_sb = xtpool.tile([128, NB], F32R)
                        nc.vector.tensor_copy(xT_sb[0:vlen, :], xT_ps[0:vlen, :])
                        # accumulate hT += w_chunk.T @ xT_chunk   (f32r, N=NB)
                        nc.tensor.matmul(
                            hT_ps[:, :],
                            w_r[0:vlen, c, :],
                            xT_sb[0:vlen, :],
                            start=(chunk_i == 0),
                            stop=(chunk_i == n_chunks_total - 1),
                        )
                        # transpose w-chunk for the decoder (psum partition 0
                        # only; walrus forbids transpose outputs elsewhere)
                        wT_ps = ps_w.tile([D, 128], F32)
                        nc.tensor.matmul(
                            wT_ps[0:D, 0:vlen],
                            w_t[0:vlen, c, :],
                            ident[0:vlen, 0:vlen],
                            is_transpose=True,
                        )
                        nc.scalar.copy(
                            wt_stage[0:D, c * 128 : c * 128 + vlen],
                            wT_ps[0:D, 0:vlen],
                        )
                        chunk_i += 1
                    # move staged wT slice to its partition group(s); a
                    # super-chunk may straddle a quarter boundary
                    seg = v0
                    while seg < v0 + sl:
                        g = seg // qb
                        seg_end = min((g + 1) * qb, v0 + sl)
                        nc.sync.dma_start(
                            wT[32 * g : 32 * g + D, seg - g * qb : seg_end - g * qb],
                            wt_stage[0:D, seg - v0 : seg_end - v0],
                        )
                        seg = seg_end

                # hT = sigmoid(hT_pre + b); round to f32r; replicate to groups
                hT_f32 = const_pool.tile([D, NB], F32)
                nc.scalar.activation(
                    hT_f32[:, :],
                    hT_ps[:, :],
                    mybir.ActivationFunctionType.Sigmoid,
                    bias=b_sb[:, 0:1],
                )
                nc.any.tensor_copy(hT_rep[0:D, :], hT_f32[:, :])
                for g in range(1, 4):
                    nc.sync.dma_start(hT_rep[32 * g : 32 * g + D, :], hT_rep[0:D, :])

            # ---------------- pass 2: decoder ----------------
            with ExitStack() as p2:
                ypool = p2.enter_context(tc.tile_pool(name="y", bufs=y_bufs))
                ps_y = p2.enter_context(tc.tile_pool(name="psy", bufs=6, space="PSUM"))
                YS = 512 * y_tile_chunks  # y sbuf tile free size
                for t in range(nbt):
                    for g in range(4):
                        q0, qlen = quarters[g]
                        if qlen == 0:
                            continue
                        for yo in range(0, qlen, YS):
                            ylen = min(YS, qlen - yo)
                            y_sb = ypool.tile([128, YS], F32)
                            for co in range(0, ylen, 512):
                                nlen = min(512, ylen - co)
                                y_ps = ps_y.tile([128, 512], F32)
                                nc.tensor.matmul(
                                    y_ps[:, 0:nlen],
                                    hT_rep[32 * g : 32 * g + D, t * 128 : (t + 1) * 128],
                                    wT[32 * g : 32 * g + D, yo + co : yo + co + nlen],
                                    tile_position=(32 * g, 0),
                                )
                                nc.scalar.activation(
                                    y_sb[:, co : co + nlen],
                                    y_ps[:, 0:nlen],
                                    mybir.ActivationFunctionType.Sigmoid,
                                )
                            nc.sync.dma_start(
                                y_d[t * 128 : (t + 1) * 128, q0 + yo : q0 + yo + ylen],
                                y_sb[:, 0:ylen],
                            )

        if repeat == 1:
            _passes()
        else:
            # timing aid: run the whole two-pass kernel `repeat` times on
            # device inside one NEFF (For_i back-edge ~2us per iteration)
            with tc.For_i(0, repeat, 1):
                _passes()

    nc.compile()
    return nc


_NC_CACHE = None


def _get_nc():
    global _NC_CACHE
    if _NC_CACHE is None:
        _NC_CACHE = build_dae(B_CORE, V)
    return _NC_CACHE


def _in_maps(x, w, b):
    x = np.ascontiguousarray(x, dtype=np.float32)
    w = np.ascontiguousarray(w, dtype=np.float32)
    b = np.ascontiguousarray(b, dtype=np.float32)
    return [
        {"x": x[i * B_CORE : (i + 1) * B_CORE], "w": w, "b": b}
        for i in range(N_CORES)
    ]


def kernel(x, w, b):
    assert x.shape == (B_FULL, V) and w.shape == (V, D) and b.shape == (D,)
    nc = _get_nc()
    in_maps = _in_maps(x, w, b)
    last = None
    # the first execution of a freshly compiled NEFF on this axon terminal
    # occasionally reports NRT_EXEC_UNIT_UNRECOVERABLE; a retry succeeds
    for _ in range(3):
        try:
            res = run_bass_kernel_spmd(nc, in_maps, core_ids=list(range(N_CORES)))
            break
        except Exception as e:  # noqa: BLE001
            last = e
    else:
        raise last
    return np.concatenate([res.results[i]["y"] for i in range(N_CORES)], axis=0)



# revision 10
# speedup vs baseline: 1.7691x; 1.7691x over previous
"""Trainium2 Bass kernel for nn_DAE_44779329028610 (embedding autoencoder).

  y = sigmoid(sigmoid(x @ w + b) @ w.T)
  x [4096, 81616] f32, w [81616, 32] f32, b [32] f32 -> y [4096, 81616] f32

Strategy: data-parallel shard of the batch dim across 8 NeuronCores
(512 rows/core); w and b replicated. The workload is HBM-bound, so all
bulk I/O is bf16: the host rounds x to bf16 (zero-padded to 81920 =
640*128 vocab cols for uniform tiling), the device writes y as bf16, and
the host upcasts at the end. Rounding error is ~2e-3 relative L2, far
under the 2e-2 gate, and halves DMA traffic vs the f32 roofline.

DMA geometry matters more than anything here: the DMA engines are
element-rate limited, not byte-rate limited (measured: f32 streams
sustain ~330 GB/s, bf16 streams only ~230 GB/s for identical descriptor
sizes), so every bulk DMA moves the bf16 bytes through f32-typed access
patterns (AP.bitcast halves the element count; DRAM tensors are declared
f32 with half the columns and the host passes bf16 arrays viewed as
f32). The kernel also uses few, large DMAs: x arrives in [128, 2, 8192]
bf16-value super-tiles (one 4 MB DMA covers a vocab range for TWO
128-row batch blocks, 16 KB per descriptor), y leaves in [128, 8192]
tiles (2 MB, 16 KB/descriptor), and the weights are loaded once per pass
in two 5 MB DMAs (host-prepacked layouts, so no on-device weight
transposes at all).

Per core the batch (512 rows) is processed as 2 super-blocks of 256 rows.
Per super-block:
  encoder: PE-transposes each [128, 128] x sub-block into [128, 1024]
  bf16 PSUM groups; DVE evicts groups to SBUF; one N=256 bf16 matmul per
  vocab chunk accumulates hT[32, 256] f32 in a PSUM bank (w-chunk
  [128, 32] stationary from the host-prepacked w_enc). ACT applies
  sigmoid+bias (-> bf16), SBUF->SBUF DMAs replicate hT to all 4
  PE row groups.
  decoder: wT comes host-pre-transposed in the 4-partition-group layout
  ([128, 20480] bf16: vocab quarter g on partitions 32g..32g+32). K=32
  matmuls in PE row group g (tile_position=(32g, 0)) fill [128, 1024]
  f32 PSUM tiles; ACT applies sigmoid -> bf16 y tiles; stores issue from
  the ACT queue (SP queue stays free for x loads).

Super-block 1's encoder (reads) overlaps super-block 0's decoder
(writes + ACT sigmoids), hiding the ACT-bound decoder tail.
"""

import sys

if "/opt/trn_rl_repo" not in sys.path:
    sys.path.insert(0, "/opt/trn_rl_repo")

from contextlib import ExitStack

import numpy as np
import ml_dtypes

from concourse import bacc, masks, mybir, tile
from concourse.bass_utils import run_bass_kernel_spmd

# The neuronx_cc hook recompiles the NEFF from scratch in every process
# (~5 min of walrus for this kernel). Cache the compiled NEFF on disk,
# keyed by the BIR hash, so repeat runs are instant.
import hashlib
import os
import shutil

import concourse.bass2jax as _bass2jax

_NEFF_CACHE_DIR = "/tmp/bass_neff_cache"
_orig_compile_bir_kernel = _bass2jax.compile_bir_kernel


def _cached_compile_bir_kernel(bir_json, tmpdir, neff_name="file.neff"):
    os.makedirs(_NEFF_CACHE_DIR, exist_ok=True)
    key = hashlib.sha256(bir_json).hexdigest()[:32]
    cpath = os.path.join(_NEFF_CACHE_DIR, f"{key}.neff")
    out = os.path.join(tmpdir, neff_name)
    if os.path.exists(cpath):
        shutil.copyfile(cpath, out)
        return out
    out = _orig_compile_bir_kernel(bir_json, tmpdir, neff_name)
    try:
        shutil.copyfile(out, cpath)
    except OSError:
        pass
    return out


_bass2jax.compile_bir_kernel = _cached_compile_bir_kernel

F32 = mybir.dt.float32
BF16 = mybir.dt.bfloat16
BF16_NP = ml_dtypes.bfloat16

B_FULL = 4096
V = 81616
D = 32
N_CORES = 8
B_CORE = B_FULL // N_CORES

V_PAD = 81920  # 640 * 128 == 4 * 20480; uniform 128-chunks + quarters
NCH = V_PAD // 128  # 640 vocab chunks
QB = V_PAD // 4  # 20480, one wT partition-group quarter
PAIR = 2  # batch blocks per super-block
XSL = 8192  # x super-tile vocab cols (4 MB DMA, 16 KB/descriptor)
YL = 8192  # y tile cols (2 MB DMA, 16 KB/descriptor)


def build_dae(B_core, V_, repeat=1):
    """Build + compile the per-core Bass program (V_ kept for test.py API)."""
    assert B_core % (PAIR * 128) == 0
    npair = B_core // (PAIR * 128)

    nc = bacc.Bacc("TRN2", target_bir_lowering=False, debug=False)

    x_d = nc.dram_tensor("x", [B_core, V_PAD // 2], F32, kind="ExternalInput")
    wenc_d = nc.dram_tensor("wenc", [128, NCH * D // 2], F32, kind="ExternalInput")
    wt4_d = nc.dram_tensor("wt4", [128, QB // 2], F32, kind="ExternalInput")
    b_d = nc.dram_tensor("b", [D], F32, kind="ExternalInput")
    y_d = nc.dram_tensor("y", [B_core, V_PAD // 2], F32, kind="ExternalOutput")

    SIG = mybir.ActivationFunctionType.Sigmoid
    NB = PAIR * 128  # 256 batch cols per super-block

    with tile.TileContext(nc) as tc, ExitStack() as ctx:
        const_pool = ctx.enter_context(tc.tile_pool(name="const", bufs=1))
        ident = const_pool.tile([128, 128], BF16)
        masks.make_identity(nc, ident[:])
        b_sb = const_pool.tile([D, 1], F32)
        nc.sync.dma_start(b_sb[:, 0:1], b_d[:].unsqueeze(1))

        def _body():
            with ExitStack() as it:
                wpool = it.enter_context(tc.tile_pool(name="w", bufs=1))
                hpool = it.enter_context(tc.tile_pool(name="h", bufs=2))
                xpool = it.enter_context(tc.tile_pool(name="x", bufs=2))
                xtpool = it.enter_context(tc.tile_pool(name="xt", bufs=4))
                ypool = it.enter_context(tc.tile_pool(name="y", bufs=2))
                psx = it.enter_context(tc.tile_pool(name="psx", bufs=3, space="PSUM"))
                psh = it.enter_context(tc.tile_pool(name="psh", bufs=1, space="PSUM"))
                psy = it.enter_context(tc.tile_pool(name="psy", bufs=2, space="PSUM"))

                w_enc = wpool.tile([128, NCH * D], BF16)
                nc.sync.dma_start(w_enc[:].bitcast(F32), wenc_d[:])
                wT4 = wpool.tile([128, QB], BF16)
                nc.sync.dma_start(wT4[:].bitcast(F32), wt4_d[:])

                for pair in range(npair):
                    pr0 = pair * NB
                    # ---------------- encoder (both blocks) ----------------
                    hT_ps = psh.tile([D, NB], F32)
                    chunk = 0
                    for v0 in range(0, V_PAD, XSL):
                        x_t = xpool.tile([128, PAIR, XSL], BF16)
                        nc.sync.dma_start(
                            x_t[:].bitcast(F32),
                            x_d[pr0 : pr0 + NB, v0 // 2 : (v0 + XSL) // 2].rearrange(
                                "(t p) v -> p t v", p=128
                            ),
                        )
                        # psx groups of 4 vocab chunks (4 * PAIR transposes)
                        for vg in range(0, XSL, 512):
                            xT_ps = psx.tile([128, 1024], BF16)
                            for cl in range(4):
                                for t in range(PAIR):
                                    nc.tensor.matmul(
                                        xT_ps[:, cl * NB + t * 128 : cl * NB + (t + 1) * 128],
                                        x_t[:, t, vg + cl * 128 : vg + (cl + 1) * 128],
                                        ident[:, 0:128],
                                        is_transpose=True,
                                    )
                            xT_sb = xtpool.tile([128, 1024], BF16)
                            nc.vector.tensor_copy(xT_sb[:], xT_ps[:])
                            for cl in range(4):
                                nc.tensor.matmul(
                                    hT_ps[:, :],
                                    w_enc[:, chunk * D : (chunk + 1) * D],
                                    xT_sb[:, cl * NB : (cl + 1) * NB],
                                    start=(chunk == 0),
                                    stop=(chunk == NCH - 1),
                                )
                                chunk += 1
                    # hT = sigmoid(hT_pre + b) -> bf16, replicated to the
                    # 4 PE row groups (partition movement needs DMA)
                    hT_sb = hpool.tile([128, NB], BF16)
                    nc.scalar.activation(hT_sb[0:D, :], hT_ps[:, :], SIG, bias=b_sb[:, 0:1])
                    for g in range(1, 4):
                        nc.scalar.dma_start(
                            hT_sb[32 * g : 32 * g + D, :], hT_sb[0:D, :]
                        )

                    # ---------------- decoder (both blocks) ----------------
                    for t in range(PAIR):
                        r0 = pr0 + t * 128
                        for w0 in range(0, V_PAD, YL):
                            y_sb = ypool.tile([128, YL], BF16)
                            for co in range(0, YL, 1024):
                                col = w0 + co
                                g = col // QB
                                cq = col - g * QB
                                y_ps = psy.tile([128, 1024], F32)
                                for cc in range(0, 1024, 512):
                                    nc.tensor.matmul(
                                        y_ps[:, cc : cc + 512],
                                        hT_sb[32 * g : 32 * g + D, t * 128 : (t + 1) * 128],
                                        wT4[32 * g : 32 * g + D, cq + cc : cq + cc + 512],
                                        tile_position=(32 * g, 0),
                                    )
                                nc.scalar.activation(
                                    y_sb[:, co : co + 1024], y_ps[:, :], SIG
                                )
                            # ACT queue: issue right after the producing
                            # sigmoids; SP queue stays free for x loads
                            nc.scalar.dma_start(
                                y_d[r0 : r0 + 128, w0 // 2 : (w0 + YL) // 2],
                                y_sb[:].bitcast(F32),
                            )

        if repeat == 1:
            _body()
        else:
            # timing aid: run the whole kernel `repeat` times on device
            # inside one NEFF (For_i back-edge ~2us per iteration)
            with tc.For_i(0, repeat, 1):
                _body()

    nc.compile()
    return nc


_NC_CACHE = None


def _get_nc():
    global _NC_CACHE
    if _NC_CACHE is None:
        _NC_CACHE = build_dae(B_CORE, V)
    return _NC_CACHE


def _prep_inputs(x, w, b):
    """Host-side dtype conversion + layout prep (shared by all cores)."""
    x = np.asarray(x, dtype=np.float32)
    w = np.asarray(w, dtype=np.float32)
    b = np.ascontiguousarray(b, dtype=np.float32)

    xp = np.zeros((B_FULL, V_PAD), dtype=BF16_NP)
    xp[:, :V] = x  # cast f32 -> bf16 on assignment

    wb = np.zeros((V_PAD, D), dtype=BF16_NP)
    wb[:V, :] = w
    # encoder stationary: w_enc[p, c*D + d] = w[c*128 + p, d]
    wenc = np.ascontiguousarray(
        wb.reshape(NCH, 128, D).transpose(1, 0, 2).reshape(128, NCH * D)
    )
    # decoder moving: wt4[32*g + d, j] = w[g*QB + j, d]
    wt4 = np.ascontiguousarray(
        wb.reshape(4, QB, D).transpose(0, 2, 1).reshape(128, QB)
    )
    return (
        xp.view(np.float32),
        wenc.view(np.float32),
        wt4.view(np.float32),
        b,
    )


def _in_maps(x, w, b):
    xp, wenc, wt4, b = _prep_inputs(x, w, b)
    return [
        {"x": xp[i * B_CORE : (i + 1) * B_CORE], "wenc": wenc, "wt4": wt4, "b": b}
        for i in range(N_CORES)
    ]


def kernel(x, w, b):
    assert x.shape == (B_FULL, V) and w.shape == (V, D) and b.shape == (D,)
    nc = _get_nc()
    in_maps = _in_maps(x, w, b)
    last = None
    # the first execution of a freshly compiled NEFF on this axon terminal
    # occasionally reports NRT_EXEC_UNIT_UNRECOVERABLE; a retry succeeds
    for _ in range(3):
        try:
            res = run_bass_kernel_spmd(nc, in_maps, core_ids=list(range(N_CORES)))
            break
        except Exception as e:  # noqa: BLE001
            last = e
    else:
        raise last
    y = np.concatenate([res.results[i]["y"] for i in range(N_CORES)], axis=0)
    y = np.ascontiguousarray(y).view(BF16_NP)  # [B_FULL, V_PAD] bf16 values
    return y[:, :V].astype(np.float32)
